# revision 18
# baseline (speedup 1.0000x reference)
"""GPT-2 transformer block on 8 trn2 NeuronCores (Bass/Tile).

Sharding: token-split with causal load balancing. Core c = 4*b + j handles
batch b and owns four 128-token tiles {15-j, 11-j, j+4, j} (slot order
s=0..3). Every core's causal attention volume is identical (slot chain caps
16/12/8/4 key tiles), so the SPMD program is uniform; per-core causality is
encoded in a small mask input. No collectives; the host scatters the 8
output slices back into place.

All matmul operands are bf16 (fp32 PSUM accumulation); the residual stream
stays fp32. Softmax uses the scoresT [sk, sq] layout: exp (no max
subtraction -- scores are bounded for this distribution), post-exp mask
multiply, denominator via a ones-column matmul accumulated across key tiles
with shrinking (prefix) widths, normalization via a K=1 broadcast matmul.
All transposes (LN strips, V rows) run on the DMA xbar, not the PE.
"""
import math
import os
import sys
import types

sys.path.insert(0, '/opt/trn_rl_repo')

import numpy as np
import ml_dtypes

BF = ml_dtypes.bfloat16


def _install_ntff_shim():
    """concourse's trace path imports antenv.axon_hooks, which this image
    lacks; give it a functional stand-in so trace=True doesn't crash."""
    try:
        import antenv.axon_hooks  # noqa: F401
        return
    except ImportError:
        pass
    try:
        import antenv
    except ImportError:
        return
    mod = types.ModuleType("antenv.axon_hooks")
    mod._hook = None

    def set_axon_ntff_profile_hook(h):
        mod._hook = h

    def get_axon_ntff_profile_hook():
        return mod._hook

    mod.set_axon_ntff_profile_hook = set_axon_ntff_profile_hook
    mod.get_axon_ntff_profile_hook = get_axon_ntff_profile_hook
    sys.modules["antenv.axon_hooks"] = mod
    antenv.axon_hooks = mod
    try:
        from trn_agent_boot.trn_boot import _ntff_profile_via_ctypes
        hook = _ntff_profile_via_ctypes('/opt/axon/libaxon_pjrt.so')
        if hook is not None:
            set_axon_ntff_profile_hook(hook)
    except Exception:
        pass


_install_ntff_shim()

import concourse.bass as bass
import concourse.tile as tile
from concourse import mybir, bass_utils

P = 128
B, S, E = 2, 2048, 2048
H, D, KH, G = 16, 128, 4, 4
F = 8192
OWN = 512                 # tokens owned per core
NE = E // P               # 16
NT = S // P               # 16 token tiles
NF = F // P               # 64
CAPS = (16, 12, 8, 4)     # key-tile chain length per slot
f32 = mybir.dt.float32
bf16 = mybir.dt.bfloat16
EXP_SCALE = 1.0 / math.sqrt(D)


def _alive(i):
    return sum(1 for c in CAPS if c > i)


def split_waits(nc, maxw=1):
    """This walrus build supports at most one sync-wait per instruction;
    hoist excess waits onto same-engine NoOps placed before the owner."""
    n = 0
    for fn in nc.m.functions:
        for blk in fn.blocks:
            new_insts = []
            for inst in blk.instructions:
                si = inst.sync_info
                if si is not None and si.on_wait and len(si.on_wait) > maxw:
                    waits = list(si.on_wait)
                    excess, keep = waits[:-maxw], waits[-maxw:]
                    for ci, w in enumerate(excess):
                        new_insts.append(mybir.InstNoOp(
                            name=f"{inst.name}-ws{ci}", engine=inst.engine,
                            sync_info=mybir.SyncInfo(on_wait=[w], on_update=[])))
                        n += 1
                    inst.sync_info = mybir.SyncInfo(
                        on_wait=keep, on_update=list(si.on_update or []))
                new_insts.append(inst)
            blk.instructions = new_insts
    return n


def _ln_to_bf16(nc, pool, x_tile, gb, bb, eps_t, zero_t, out_bf16):
    """LayerNorm along the free dim (E) of x_tile [P, E] fp32 -> bf16."""
    Sqrt = mybir.ActivationFunctionType.Sqrt
    Ident = mybir.ActivationFunctionType.Identity
    mult = mybir.AluOpType.mult
    add = mybir.AluOpType.add
    sub = mybir.AluOpType.subtract
    stats = pool.tile([P, E // 512, 6], f32, tag="ln_stats", bufs=2)
    for i in range(E // 512):
        nc.vector.bn_stats(out=stats[:, i, :], in_=x_tile[:, i * 512:(i + 1) * 512])
    mv = pool.tile([P, 2], f32, tag="ln_mv", bufs=2)
    nc.vector.bn_aggr(out=mv, in_=stats)
    rstd = pool.tile([P, 1], f32, tag="ln_rstd", bufs=2)
    nc.scalar.activation(out=rstd, in_=mv[:, 1:2], func=Sqrt, bias=eps_t)
    nc.vector.reciprocal(out=rstd, in_=rstd)
    nmr = pool.tile([P, 1], f32, tag="ln_nmr", bufs=2)
    nc.vector.tensor_tensor(out=nmr, in0=mv[:, 0:1], in1=rstd, op=mult)
    nc.vector.tensor_tensor(out=nmr, in0=zero_t, in1=nmr, op=sub)
    nc.scalar.activation(out=out_bf16, in_=x_tile, func=Ident,
                         scale=rstd, bias=nmr)
    nc.vector.tensor_tensor(out=out_bf16, in0=out_bf16, in1=gb, op=mult)
    nc.vector.tensor_tensor(out=out_bf16, in0=out_bf16, in1=bb, op=add)


def build():
    nc = bass.Bass("TRN2", target_bir_lowering=False, debug=False, num_devices=8)

    t_ = {}
    t_["xkv"] = nc.dram_tensor("xkv", [S, E], f32, kind="ExternalInput").ap()
    t_["xow"] = nc.dram_tensor("xow", [OWN, E], f32, kind="ExternalInput").ap()
    t_["masks"] = nc.dram_tensor("masks", [P, NT, OWN], bf16, kind="ExternalInput").ap()
    t_["wq_s"] = nc.dram_tensor("wq_s", [H, P, NE, P], bf16, kind="ExternalInput").ap()
    t_["wk_s"] = nc.dram_tensor("wk_s", [KH, P, NE, P], bf16, kind="ExternalInput").ap()
    t_["wv_s"] = nc.dram_tensor("wv_s", [KH, P, NE, P], bf16, kind="ExternalInput").ap()
    t_["wo_t"] = nc.dram_tensor("wo_t", [H, 4, P, 512], bf16, kind="ExternalInput").ap()
    t_["wu_s"] = nc.dram_tensor("wu_s", [NF, P, NE, P], bf16, kind="ExternalInput").ap()
    t_["wd_t"] = nc.dram_tensor("wd_t", [NF, 4, P, 512], bf16, kind="ExternalInput").ap()
    t_["bq"] = nc.dram_tensor("bq", [E], f32, kind="ExternalInput").ap()
    t_["bk"] = nc.dram_tensor("bk", [KH * D], f32, kind="ExternalInput").ap()
    t_["bv"] = nc.dram_tensor("bv", [KH * D], f32, kind="ExternalInput").ap()
    t_["bo"] = nc.dram_tensor("bo", [E], f32, kind="ExternalInput").ap()
    t_["bu"] = nc.dram_tensor("bu", [F], f32, kind="ExternalInput").ap()
    t_["bd"] = nc.dram_tensor("bd", [E], f32, kind="ExternalInput").ap()
    t_["g1"] = nc.dram_tensor("g1", [E], bf16, kind="ExternalInput").ap()
    t_["b1"] = nc.dram_tensor("b1", [E], bf16, kind="ExternalInput").ap()
    t_["g2"] = nc.dram_tensor("g2", [E], bf16, kind="ExternalInput").ap()
    t_["b2"] = nc.dram_tensor("b2", [E], bf16, kind="ExternalInput").ap()
    t_["out"] = nc.dram_tensor("out", [OWN, E], f32, kind="ExternalOutput").ap()

    with tile.TileContext(nc) as tc:
        _build_body(nc, tc, t_)
    return nc


def _build_body(nc, tc, t_):
    xkv, xow, maskd = t_["xkv"], t_["xow"], t_["masks"]
    wq_s, wk_s, wv_s = t_["wq_s"], t_["wk_s"], t_["wv_s"]
    wo_t, wu_s, wd_t = t_["wo_t"], t_["wu_s"], t_["wd_t"]
    bq, bk, bv, bo, bu, bd = (t_[k] for k in ("bq", "bk", "bv", "bo", "bu", "bd"))
    g1, b1, g2, b2, out = (t_[k] for k in ("g1", "b1", "g2", "b2", "out"))
    Ident = mybir.ActivationFunctionType.Identity
    Exp = mybir.ActivationFunctionType.Exp
    Gelu = mybir.ActivationFunctionType.Gelu
    mult = mybir.AluOpType.mult
    add = mybir.AluOpType.add

    with tc.tile_pool(name="persist", bufs=1) as persist:
        eps_t = persist.tile([P, 1], f32)
        nc.vector.memset(eps_t, 1e-5)
        zero_t = persist.tile([P, 1], f32)
        nc.vector.memset(zero_t, 0.0)
        ones_col = persist.tile([P, 1], bf16)   # lhsT for denominator (K=P, M=1)
        nc.vector.memset(ones_col, 1.0)
        ones_row = persist.tile([1, P], bf16)   # lhsT for broadcast (K=1, M=P)
        nc.vector.memset(ones_row, 1.0)
        bq_sb = persist.tile([P, H], f32)
        nc.sync.dma_start(out=bq_sb, in_=bq.rearrange("(t p) -> p t", p=P))
        bk_sb = persist.tile([P, KH], f32)
        nc.sync.dma_start(out=bk_sb, in_=bk.rearrange("(t p) -> p t", p=P))
        bv_sb = persist.tile([P, KH], f32)
        nc.sync.dma_start(out=bv_sb, in_=bv.rearrange("(t p) -> p t", p=P))
        bu_sb = persist.tile([P, NF], f32)
        nc.sync.dma_start(out=bu_sb, in_=bu.rearrange("(t p) -> p t", p=P))
        g1_b = persist.tile([P, E], bf16)
        nc.sync.dma_start(out=g1_b, in_=g1.unsqueeze(0).to_broadcast((P, E)))
        b1_b = persist.tile([P, E], bf16)
        nc.sync.dma_start(out=b1_b, in_=b1.unsqueeze(0).to_broadcast((P, E)))

        with tc.tile_pool(name="midkeep", bufs=1) as midkeep:
            with tc.tile_pool(name="qkv_keep", bufs=1) as qkv_keep:
                qT_all = qkv_keep.tile([P, H, OWN], bf16, name="qT_all")
                kT = [qkv_keep.tile([P, S], bf16, tag=f"kT{i}", name=f"kT{i}")
                      for i in range(KH)]
                vtok = qkv_keep.tile([P, NT, KH * P], bf16, name="vtok")

                # ---------- Phase A: LN1 + Q/K/V projections ----------
                with (
                    tc.tile_pool(name="pA", bufs=1) as pA,
                    tc.tile_pool(name="psA", bufs=1, space="PSUM") as psA,
                ):
                    x1ownT = pA.tile([P, NE, OWN], bf16, name="x1ownT")
                    # LN1 of own tokens (slot order) -> x1ownT
                    for ms in range(4):
                        xo_t = pA.tile([P, E], f32, tag="xo", bufs=2)
                        nc.scalar.dma_start(out=xo_t, in_=xow[ms * P:(ms + 1) * P, :])
                        x1b = pA.tile([P, E], bf16, tag="x1b", bufs=2)
                        _ln_to_bf16(nc, pA, xo_t, g1_b, b1_b, eps_t, zero_t, x1b)
                        nc.scalar.dma_start(
                            out=x1ownT[:, :, ms * P:(ms + 1) * P], in_=x1b,
                            transpose=True)
                    # Q projections (PE starts here)
                    for m in range(H):
                        wstrip = pA.tile([P, NE, P], bf16, tag="wq", bufs=3)
                        nc.sync.dma_start(out=wstrip, in_=wq_s[m])
                        psq = psA.tile([P, OWN], f32, tag="psq", bufs=2)
                        for e in range(NE):
                            nc.tensor.matmul(psq, wstrip[:, e, :], x1ownT[:, e, :],
                                             start=(e == 0), stop=(e == NE - 1))
                        nc.scalar.activation(out=qT_all[:, m, :], in_=psq, func=Ident,
                                             bias=bq_sb[:, m:m + 1])
                    # K/V for the full sequence, in quarters of 512 tokens
                    wkst = []
                    wvst = []
                    for kv in range(KH):
                        wk_t = pA.tile([P, NE, P], bf16, tag=f"wk{kv}", name=f"wk{kv}")
                        nc.sync.dma_start(out=wk_t, in_=wk_s[kv])
                        wkst.append(wk_t)
                        wv_t = pA.tile([P, NE, P], bf16, tag=f"wv{kv}", name=f"wv{kv}")
                        nc.sync.dma_start(out=wv_t, in_=wv_s[kv])
                        wvst.append(wv_t)
                    for q in range(4):
                        x1Tq = pA.tile([P, NE, 512], bf16, tag="x1Tq", bufs=2)
                        for tt in range(4):
                            t = q * 4 + tt
                            x_t = pA.tile([P, E], f32, tag="xo", bufs=2)
                            nc.scalar.dma_start(out=x_t, in_=xkv[t * P:(t + 1) * P, :])
                            x1b = pA.tile([P, E], bf16, tag="x1b", bufs=2)
                            _ln_to_bf16(nc, pA, x_t, g1_b, b1_b, eps_t, zero_t, x1b)
                            nc.scalar.dma_start(
                                out=x1Tq[:, :, tt * P:(tt + 1) * P], in_=x1b,
                                transpose=True)
                        for kv in range(KH):
                            pskv = psA.tile([P, 512], f32, tag="pskv", bufs=2)
                            for e in range(NE):
                                nc.tensor.matmul(pskv, wkst[kv][:, e, :],
                                                 x1Tq[:, e, :],
                                                 start=(e == 0), stop=(e == NE - 1))
                            nc.scalar.activation(
                                out=kT[kv][:, q * 512:(q + 1) * 512], in_=pskv,
                                func=Ident, bias=bk_sb[:, kv:kv + 1])
                            psv = psA.tile([P, 512], f32, tag="pskv", bufs=2)
                            for e in range(NE):
                                nc.tensor.matmul(psv, wvst[kv][:, e, :],
                                                 x1Tq[:, e, :],
                                                 start=(e == 0), stop=(e == NE - 1))
                            vrow = pA.tile([P, 512], bf16, tag="vrow", bufs=2)
                            nc.scalar.activation(out=vrow, in_=psv, func=Ident,
                                                 bias=bv_sb[:, kv:kv + 1])
                            nc.scalar.dma_start(
                                out=vtok[:, q * 4:(q + 1) * 4, kv * P:(kv + 1) * P],
                                in_=vrow, transpose=True)

                # ---------- Phase B/C: attention + o-proj ----------
                xmid = [midkeep.tile([P, E], f32, tag=f"xmid{i}", name=f"xmid{i}")
                        for i in range(4)]
                with tc.tile_pool(name="oT_keep", bufs=1) as oT_keep:
                    oT_all = oT_keep.tile([P, H, OWN], bf16, name="oT_all")
                    with (
                        tc.tile_pool(name="pB", bufs=1) as pB,
                        tc.tile_pool(name="psB", bufs=1, space="PSUM") as psB,
                    ):
                        masks = pB.tile([P, NT, OWN], bf16, name="masks")
                        nc.sync.dma_start(out=masks, in_=maskd)
                        # two heads sharing a kv head run interleaved so the
                        # PE->ACT->DVE->PE softmax round-trip of one hides the
                        # other; consecutive matmuls also share lhsT.
                        for g in range(H // 2):
                            hs = (2 * g, 2 * g + 1)
                            kv = hs[0] // G
                            ps_o = [psB.tile([P, OWN], f32, tag=f"ps_o{u}",
                                             bufs=1, name=f"ps_o{u}")
                                    for u in range(2)]
                            ps_den = psB.tile([1, 2, OWN], f32, tag="ps_den",
                                              bufs=1)
                            for i in range(NT):
                                w = P * _alive(i)
                                # each matmul writes within one PSUM bank; the
                                # paired exp reads both banks in one pass
                                ps_s = psB.tile([P, 2, OWN], f32, tag="ps_s",
                                                bufs=2)
                                for u in range(2):
                                    nc.tensor.matmul(ps_s[:, u, 0:w],
                                                     kT[kv][:, i * P:(i + 1) * P],
                                                     qT_all[:, hs[u], 0:w],
                                                     start=True, stop=True)
                                ex = pB.tile([P, 2, OWN], bf16, tag="ex", bufs=3)
                                nc.scalar.activation(out=ex[:, :, 0:w],
                                                     in_=ps_s[:, :, 0:w],
                                                     func=Exp, scale=EXP_SCALE)
                                for u in range(2):
                                    nc.vector.tensor_tensor(out=ex[:, u, 0:w],
                                                            in0=ex[:, u, 0:w],
                                                            in1=masks[:, i, 0:w],
                                                            op=mult)
                                for u in range(2):
                                    nc.tensor.matmul(ps_o[u][:, 0:w],
                                                     vtok[:, i, kv * P:(kv + 1) * P],
                                                     ex[:, u, 0:w],
                                                     start=(i == 0),
                                                     stop=(i == NT - 1))
                                for u in range(2):
                                    nc.tensor.matmul(ps_den[:, u, 0:w], ones_col,
                                                     ex[:, u, 0:w],
                                                     start=(i == 0),
                                                     stop=(i == NT - 1))
                            rden = pB.tile([1, 2, OWN], bf16, tag="rden", bufs=2)
                            with nc.allow_low_precision(
                                    reason="softmax denominator"):
                                nc.vector.reciprocal(out=rden, in_=ps_den)
                            ps_bc = psB.tile([P, 2, OWN], f32, tag="ps_s", bufs=2)
                            for u in range(2):
                                nc.tensor.matmul(ps_bc[:, u, :], ones_row,
                                                 rden[:, u, :],
                                                 start=True, stop=True)
                            bc = pB.tile([P, 2, OWN], f32, tag="bc", bufs=2)
                            nc.vector.tensor_copy(bc, ps_bc)
                            for u in range(2):
                                nc.vector.tensor_tensor(out=oT_all[:, hs[u], :],
                                                        in0=ps_o[u],
                                                        in1=bc[:, u, :],
                                                        op=mult)

                    with (
                        tc.tile_pool(name="pC", bufs=1) as pC,
                        tc.tile_pool(name="psC", bufs=1, space="PSUM") as psC,
                    ):
                        bo_b = pC.tile([P, E], f32, tag="bo_b")
                        nc.sync.dma_start(out=bo_b,
                                          in_=bo.unsqueeze(0).to_broadcast((P, E)))
                        for ms in range(4):
                            nc.scalar.dma_start(out=xmid[ms],
                                              in_=xow[ms * P:(ms + 1) * P, :])
                            nc.vector.tensor_tensor(out=xmid[ms], in0=xmid[ms],
                                                    in1=bo_b, op=add)
                        for ec in range(4):
                            pso = [psC.tile([P, 512], f32, tag=f"pso{i}", bufs=2,
                                            name=f"pso{i}") for i in range(4)]
                            for k in range(H):
                                wtile = pC.tile([P, 512], bf16, tag="wo", bufs=4)
                                nc.sync.dma_start(out=wtile, in_=wo_t[k, ec])
                                for ms in range(4):
                                    nc.tensor.matmul(
                                        pso[ms],
                                        oT_all[:, k, ms * P:(ms + 1) * P],
                                        wtile, start=(k == 0),
                                        stop=(k == H - 1))
                            for ms in range(4):
                                nc.vector.tensor_tensor(
                                    out=xmid[ms][:, ec * 512:(ec + 1) * 512],
                                    in0=pso[ms],
                                    in1=xmid[ms][:, ec * 512:(ec + 1) * 512], op=add)

            # ---------- Phase D/E: LN2, MLP ----------
            with (
                tc.tile_pool(name="pM", bufs=1) as pM,
                tc.tile_pool(name="psM", bufs=1, space="PSUM") as psM,
            ):
                g2_b = pM.tile([P, E], bf16, tag="g2b")
                nc.sync.dma_start(out=g2_b, in_=g2.unsqueeze(0).to_broadcast((P, E)))
                b2_b = pM.tile([P, E], bf16, tag="b2b")
                nc.sync.dma_start(out=b2_b, in_=b2.unsqueeze(0).to_broadcast((P, E)))
                bd_b = pM.tile([P, E], f32, tag="bd_b")
                nc.sync.dma_start(out=bd_b, in_=bd.unsqueeze(0).to_broadcast((P, E)))
                x2T = pM.tile([P, NE, OWN], bf16, name="x2T")
                for ms in range(4):
                    x2b = pM.tile([P, E], bf16, tag="x2b", bufs=2)
                    _ln_to_bf16(nc, pM, xmid[ms], g2_b, b2_b, eps_t, zero_t, x2b)
                    nc.scalar.dma_start(out=x2T[:, :, ms * P:(ms + 1) * P], in_=x2b,
                                      transpose=True)
                    # fold the final bias into the residual so the tail is
                    # a single add per chunk
                    nc.vector.tensor_tensor(out=xmid[ms], in0=xmid[ms],
                                            in1=bd_b, op=add)
                outp = [pM.tile([P, E], f32, tag=f"outp{i}", name=f"outp{i}")
                        for i in range(4)]
                hT = [pM.tile([P, OWN], bf16, tag=f"hT{i}", name=f"hT{i}")
                      for i in range(NF // 2)]
                for fh in range(2):
                    for fi in range(NF // 2):
                        f = fh * (NF // 2) + fi
                        wstrip = pM.tile([P, NE, P], bf16, tag="wu", bufs=3)
                        nc.sync.dma_start(out=wstrip, in_=wu_s[f])
                        psh = psM.tile([P, OWN], f32, tag="psh", bufs=2)
                        for e in range(NE):
                            nc.tensor.matmul(psh, wstrip[:, e, :], x2T[:, e, :],
                                             start=(e == 0), stop=(e == NE - 1))
                        nc.scalar.activation(out=hT[fi], in_=psh, func=Gelu,
                                             bias=bu_sb[:, f:f + 1])
                    for ec in range(4):
                        psd = [psM.tile([P, 512], f32, tag=f"psd{i}", bufs=1,
                                        name=f"psd{i}") for i in range(4)]
                        for fi in range(NF // 2):
                            f = fh * (NF // 2) + fi
                            wtile = pM.tile([P, 512], bf16, tag="wd", bufs=4)
                            nc.sync.dma_start(out=wtile, in_=wd_t[f, ec])
                            for ms in range(4):
                                nc.tensor.matmul(psd[ms],
                                                 hT[fi][:, ms * P:(ms + 1) * P],
                                                 wtile, start=(fi == 0),
                                                 stop=(fi == NF // 2 - 1))
                        cols = slice(ec * 512, (ec + 1) * 512)
                        for ms in range(4):
                            if fh == 0:
                                nc.vector.tensor_copy(outp[ms][:, cols], psd[ms])
                            else:
                                nc.vector.tensor_tensor(
                                    out=outp[ms][:, cols], in0=psd[ms],
                                    in1=outp[ms][:, cols], op=add)
                                nc.vector.tensor_tensor(
                                    out=outp[ms][:, cols], in0=outp[ms][:, cols],
                                    in1=xmid[ms][:, cols], op=add)
                                nc.sync.dma_start(
                                    out=out[ms * P:(ms + 1) * P, cols],
                                    in_=outp[ms][:, cols])


_NC_CACHE = None
LAST_RESULTS = None


def _get_nc():
    global _NC_CACHE
    if _NC_CACHE is None:
        nc = build()
        split_waits(nc)
        _NC_CACHE = nc
    return _NC_CACHE


def _slot_tiles(j):
    return [15 - j, 11 - j, j + 4, j]


def _prep_shared(wq, wk, wv, wo, wu, wd):
    def strips(w, n):  # [E, n*128] -> [n, 128(p), NE(t), 128(m)] contiguous
        a = np.asarray(w, np.float32).reshape(NE, P, n, P)
        return np.ascontiguousarray(a.transpose(2, 1, 0, 3)).astype(BF)

    def tiles(w, nr):  # [nr*128, E] -> [nr, 4, 128, 512]
        a = np.asarray(w, np.float32).reshape(nr, P, 4, 512)
        return np.ascontiguousarray(a.transpose(0, 2, 1, 3)).astype(BF)

    return {
        "wq_s": strips(wq, H),
        "wk_s": strips(wk, KH),
        "wv_s": strips(wv, KH),
        "wo_t": tiles(wo, H),
        "wu_s": strips(wu, NF),
        "wd_t": tiles(wd, NF),
    }


def _make_masks(j):
    m = np.zeros((P, NT, OWN), np.float32)
    ki = np.arange(P)[:, None]
    qi = np.arange(P)[None, :]
    tri = (ki <= qi).astype(np.float32)
    for s, t in enumerate(_slot_tiles(j)):
        for i in range(CAPS[s]):
            if i < t:
                m[:, i, s * P:(s + 1) * P] = 1.0
            elif i == t:
                m[:, i, s * P:(s + 1) * P] = tri
    return m.astype(BF)


def kernel(x, ln1_g, ln1_b, wq, bq, wk, bk, wv, bv, wo, bo, ln2_g, ln2_b,
           wu, bu, wd, bd):
    x = np.asarray(x, np.float32)
    shared = _prep_shared(wq, wk, wv, wo, wu, wd)
    shared.update({
        "bq": np.asarray(bq, np.float32), "bk": np.asarray(bk, np.float32),
        "bv": np.asarray(bv, np.float32), "bo": np.asarray(bo, np.float32),
        "bu": np.asarray(bu, np.float32), "bd": np.asarray(bd, np.float32),
        "g1": np.asarray(ln1_g, np.float32).astype(BF),
        "b1": np.asarray(ln1_b, np.float32).astype(BF),
        "g2": np.asarray(ln2_g, np.float32).astype(BF),
        "b2": np.asarray(ln2_b, np.float32).astype(BF),
    })
    in_maps = []
    for core in range(8):
        b, j = divmod(core, 4)
        m = dict(shared)
        m["xkv"] = np.ascontiguousarray(x[b])
        m["xow"] = np.ascontiguousarray(np.concatenate(
            [x[b, t * P:(t + 1) * P] for t in _slot_tiles(j)], axis=0))
        m["masks"] = _make_masks(j)
        in_maps.append(m)

    nc = _get_nc()
    trace = bool(os.environ.get("KERNEL_TRACE"))
    res = bass_utils.run_bass_kernel_spmd(
        nc, in_maps, core_ids=list(range(8)), trace=trace)
    global LAST_RESULTS
    LAST_RESULTS = res
    out = np.empty((B, S, E), np.float32)
    for core in range(8):
        b, j = divmod(core, 4)
        r = res.results[core]["out"]
        for s, t in enumerate(_slot_tiles(j)):
            out[b, t * P:(t + 1) * P] = r[s * P:(s + 1) * P]
    return out


# revision 25
# speedup vs baseline: 1.1103x; 1.1103x over previous
"""GPT-2 transformer block on 8 trn2 NeuronCores (Bass/Tile).

Sharding: token-split with causal load balancing. Core c = 4*b + j handles
batch b and owns four 128-token tiles {15-j, 11-j, j+4, j} (slot order
s=0..3). Every core's causal attention volume is identical (slot chain caps
16/12/8/4 key tiles), so the SPMD program is uniform; per-core causality is
encoded in a small mask input. No collectives; the host scatters the 8
output slices back into place.

All matmul operands are bf16 (fp32 PSUM accumulation); the residual stream
stays fp32. Softmax uses the scoresT [sk, sq] layout: exp (no max
subtraction -- scores are bounded for this distribution), post-exp mask
multiply, denominator via a ones-column matmul accumulated across key tiles
with shrinking (prefix) widths, normalization via a K=1 broadcast matmul.
All transposes (LN strips, V rows) run on the DMA xbar, not the PE.
"""
import math
import os
import sys
import types

sys.path.insert(0, '/opt/trn_rl_repo')

import numpy as np
import ml_dtypes

BF = ml_dtypes.bfloat16


def _install_ntff_shim():
    """concourse's trace path imports antenv.axon_hooks, which this image
    lacks; give it a functional stand-in so trace=True doesn't crash."""
    try:
        import antenv.axon_hooks  # noqa: F401
        return
    except ImportError:
        pass
    try:
        import antenv
    except ImportError:
        return
    mod = types.ModuleType("antenv.axon_hooks")
    mod._hook = None

    def set_axon_ntff_profile_hook(h):
        mod._hook = h

    def get_axon_ntff_profile_hook():
        return mod._hook

    mod.set_axon_ntff_profile_hook = set_axon_ntff_profile_hook
    mod.get_axon_ntff_profile_hook = get_axon_ntff_profile_hook
    sys.modules["antenv.axon_hooks"] = mod
    antenv.axon_hooks = mod
    try:
        from trn_agent_boot.trn_boot import _ntff_profile_via_ctypes
        hook = _ntff_profile_via_ctypes('/opt/axon/libaxon_pjrt.so')
        if hook is not None:
            set_axon_ntff_profile_hook(hook)
    except Exception:
        pass


_install_ntff_shim()

import concourse.bass as bass
import concourse.tile as tile
from concourse import mybir, bass_utils

P = 128
B, S, E = 2, 2048, 2048
H, D, KH, G = 16, 128, 4, 4
F = 8192
OWN = 512                 # tokens owned per core
NE = E // P               # 16
NT = S // P               # 16 token tiles
NF = F // P               # 64
CAPS = (16, 12, 8, 4)     # key-tile chain length per slot
f32 = mybir.dt.float32
bf16 = mybir.dt.bfloat16
EXP_SCALE = 1.0 / math.sqrt(D)


def _alive(i):
    return sum(1 for c in CAPS if c > i)


def split_waits(nc, maxw=1):
    """This walrus build supports at most one sync-wait per instruction;
    hoist excess waits onto same-engine NoOps placed before the owner."""
    n = 0
    for fn in nc.m.functions:
        for blk in fn.blocks:
            new_insts = []
            for inst in blk.instructions:
                si = inst.sync_info
                if si is not None and si.on_wait and len(si.on_wait) > maxw:
                    waits = list(si.on_wait)
                    excess, keep = waits[:-maxw], waits[-maxw:]
                    for ci, w in enumerate(excess):
                        new_insts.append(mybir.InstNoOp(
                            name=f"{inst.name}-ws{ci}", engine=inst.engine,
                            sync_info=mybir.SyncInfo(on_wait=[w], on_update=[])))
                        n += 1
                    inst.sync_info = mybir.SyncInfo(
                        on_wait=keep, on_update=list(si.on_update or []))
                new_insts.append(inst)
            blk.instructions = new_insts
    return n


def _ln_to_bf16(nc, pool, x_tile, eps_t, zero_t, out_bf16):
    """LayerNorm (no affine; folded into the weights) of x_tile [P, E] fp32
    -> bf16."""
    Sqrt = mybir.ActivationFunctionType.Sqrt
    Ident = mybir.ActivationFunctionType.Identity
    mult = mybir.AluOpType.mult
    sub = mybir.AluOpType.subtract
    stats = pool.tile([P, E // 512, 6], f32, tag="ln_stats", bufs=2)
    for i in range(E // 512):
        nc.vector.bn_stats(out=stats[:, i, :], in_=x_tile[:, i * 512:(i + 1) * 512])
    mv = pool.tile([P, 2], f32, tag="ln_mv", bufs=2)
    nc.vector.bn_aggr(out=mv, in_=stats)
    rstd = pool.tile([P, 1], f32, tag="ln_rstd", bufs=2)
    nc.scalar.activation(out=rstd, in_=mv[:, 1:2], func=Sqrt, bias=eps_t)
    nc.vector.reciprocal_approx_fast(out=rstd, in_=rstd)
    nmr = pool.tile([P, 1], f32, tag="ln_nmr", bufs=2)
    nc.vector.tensor_tensor(out=nmr, in0=mv[:, 0:1], in1=rstd, op=mult)
    nc.vector.tensor_tensor(out=nmr, in0=zero_t, in1=nmr, op=sub)
    nc.scalar.activation(out=out_bf16, in_=x_tile, func=Ident,
                         scale=rstd, bias=nmr)


def build():
    nc = bass.Bass("TRN2", target_bir_lowering=False, debug=False, num_devices=8)

    t_ = {}
    t_["xkv"] = nc.dram_tensor("xkv", [S, E], f32, kind="ExternalInput").ap()
    t_["xow"] = nc.dram_tensor("xow", [OWN, E], f32, kind="ExternalInput").ap()
    t_["masks"] = nc.dram_tensor("masks", [P, NT, OWN], bf16, kind="ExternalInput").ap()
    t_["wq_s"] = nc.dram_tensor("wq_s", [H, P, NE, P], bf16, kind="ExternalInput").ap()
    t_["wk_s"] = nc.dram_tensor("wk_s", [KH, P, NE, P], bf16, kind="ExternalInput").ap()
    t_["wv_s"] = nc.dram_tensor("wv_s", [KH, P, NE, P], bf16, kind="ExternalInput").ap()
    t_["wo_t"] = nc.dram_tensor("wo_t", [H, 4, P, 512], bf16, kind="ExternalInput").ap()
    t_["wu_s"] = nc.dram_tensor("wu_s", [NF, P, NE, P], bf16, kind="ExternalInput").ap()
    t_["wd_t"] = nc.dram_tensor("wd_t", [NF, 4, P, 512], bf16, kind="ExternalInput").ap()
    t_["bq"] = nc.dram_tensor("bq", [E], f32, kind="ExternalInput").ap()
    t_["bk"] = nc.dram_tensor("bk", [KH * D], f32, kind="ExternalInput").ap()
    t_["bv"] = nc.dram_tensor("bv", [KH * D], f32, kind="ExternalInput").ap()
    t_["bo"] = nc.dram_tensor("bo", [E], f32, kind="ExternalInput").ap()
    t_["bu"] = nc.dram_tensor("bu", [F], f32, kind="ExternalInput").ap()
    t_["bd"] = nc.dram_tensor("bd", [E], f32, kind="ExternalInput").ap()
    t_["out"] = nc.dram_tensor("out", [OWN, E], f32, kind="ExternalOutput").ap()

    with tile.TileContext(nc) as tc:
        _build_body(nc, tc, t_)
    return nc


def _build_body(nc, tc, t_):
    xkv, xow, maskd = t_["xkv"], t_["xow"], t_["masks"]
    wq_s, wk_s, wv_s = t_["wq_s"], t_["wk_s"], t_["wv_s"]
    wo_t, wu_s, wd_t = t_["wo_t"], t_["wu_s"], t_["wd_t"]
    out = t_["out"]
    bq, bk, bv, bo, bu, bd = (t_[k] for k in ("bq", "bk", "bv", "bo", "bu", "bd"))
    Ident = mybir.ActivationFunctionType.Identity
    Exp = mybir.ActivationFunctionType.Exp
    Gelu = mybir.ActivationFunctionType.Gelu
    mult = mybir.AluOpType.mult
    add = mybir.AluOpType.add

    with tc.tile_pool(name="persist", bufs=1) as persist:
        eps_t = persist.tile([P, 1], f32)
        nc.vector.memset(eps_t, 1e-5)
        zero_t = persist.tile([P, 1], f32)
        nc.vector.memset(zero_t, 0.0)
        ones_col = persist.tile([P, 1], bf16)   # lhsT for denominator (K=P, M=1)
        nc.vector.memset(ones_col, 1.0)
        ones_row = persist.tile([1, P], bf16)   # lhsT for broadcast (K=1, M=P)
        nc.vector.memset(ones_row, 1.0)
        bq_sb = persist.tile([P, H], f32)
        nc.sync.dma_start(out=bq_sb, in_=bq.rearrange("(t p) -> p t", p=P))
        bk_sb = persist.tile([P, KH], f32)
        nc.sync.dma_start(out=bk_sb, in_=bk.rearrange("(t p) -> p t", p=P))
        bv_sb = persist.tile([P, KH], f32)
        nc.sync.dma_start(out=bv_sb, in_=bv.rearrange("(t p) -> p t", p=P))
        bu_sb = persist.tile([P, NF], f32)
        nc.sync.dma_start(out=bu_sb, in_=bu.rearrange("(t p) -> p t", p=P))

        with tc.tile_pool(name="midkeep", bufs=1) as midkeep:
            with tc.tile_pool(name="qkv_keep", bufs=1) as qkv_keep:
                qT_all = qkv_keep.tile([P, H, OWN], bf16, name="qT_all")
                kT = [qkv_keep.tile([P, S], bf16, tag=f"kT{i}", name=f"kT{i}")
                      for i in range(KH)]
                vtok = qkv_keep.tile([P, NT, KH * P], bf16, name="vtok")

                # ---------- Phase A: LN1 + Q/K/V projections ----------
                with (
                    tc.tile_pool(name="pA", bufs=1) as pA,
                    tc.tile_pool(name="psA", bufs=1, space="PSUM") as psA,
                ):
                    x1ownT = pA.tile([P, NE, OWN], bf16, name="x1ownT")
                    # LN1 of own tokens (slot order) -> x1ownT
                    for ms in range(4):
                        xo_t = pA.tile([P, E], f32, tag="xo", bufs=2)
                        nc.scalar.dma_start(out=xo_t, in_=xow[ms * P:(ms + 1) * P, :])
                        x1b = pA.tile([P, E], bf16, tag="x1b", bufs=2)
                        _ln_to_bf16(nc, pA, xo_t, eps_t, zero_t, x1b)
                        nc.scalar.dma_start(
                            out=x1ownT[:, :, ms * P:(ms + 1) * P], in_=x1b,
                            transpose=True)
                    # Q projections (PE starts here)
                    for m in range(H):
                        wstrip = pA.tile([P, NE, P], bf16, tag="wq", bufs=3)
                        nc.sync.dma_start(out=wstrip, in_=wq_s[m])
                        psq = psA.tile([P, OWN], f32, tag="psq", bufs=2)
                        for e in range(NE):
                            nc.tensor.matmul(psq, wstrip[:, e, :], x1ownT[:, e, :],
                                             start=(e == 0), stop=(e == NE - 1))
                        nc.scalar.activation(out=qT_all[:, m, :], in_=psq, func=Ident,
                                             bias=bq_sb[:, m:m + 1])
                    # K/V for the full sequence, in quarters of 512 tokens
                    wkst = []
                    wvst = []
                    for kv in range(KH):
                        wk_t = pA.tile([P, NE, P], bf16, tag=f"wk{kv}", name=f"wk{kv}")
                        nc.sync.dma_start(out=wk_t, in_=wk_s[kv])
                        wkst.append(wk_t)
                        wv_t = pA.tile([P, NE, P], bf16, tag=f"wv{kv}", name=f"wv{kv}")
                        nc.sync.dma_start(out=wv_t, in_=wv_s[kv])
                        wvst.append(wv_t)
                    for q in range(4):
                        x1Tq = pA.tile([P, NE, 512], bf16, tag="x1Tq", bufs=2)
                        for tt in range(4):
                            t = q * 4 + tt
                            x_t = pA.tile([P, E], f32, tag="xo", bufs=2)
                            nc.scalar.dma_start(out=x_t, in_=xkv[t * P:(t + 1) * P, :])
                            x1b = pA.tile([P, E], bf16, tag="x1b", bufs=2)
                            _ln_to_bf16(nc, pA, x_t, eps_t, zero_t, x1b)
                            nc.scalar.dma_start(
                                out=x1Tq[:, :, tt * P:(tt + 1) * P], in_=x1b,
                                transpose=True)
                        for kv in range(KH):
                            pskv = psA.tile([P, 512], f32, tag="pskv", bufs=2)
                            for e in range(NE):
                                nc.tensor.matmul(pskv, wkst[kv][:, e, :],
                                                 x1Tq[:, e, :],
                                                 start=(e == 0), stop=(e == NE - 1))
                            nc.scalar.activation(
                                out=kT[kv][:, q * 512:(q + 1) * 512], in_=pskv,
                                func=Ident, bias=bk_sb[:, kv:kv + 1])
                            psv = psA.tile([P, 512], f32, tag="pskv", bufs=2)
                            for e in range(NE):
                                nc.tensor.matmul(psv, wvst[kv][:, e, :],
                                                 x1Tq[:, e, :],
                                                 start=(e == 0), stop=(e == NE - 1))
                            vrow = pA.tile([P, 512], bf16, tag="vrow", bufs=2)
                            nc.scalar.activation(out=vrow, in_=psv, func=Ident,
                                                 bias=bv_sb[:, kv:kv + 1])
                            nc.scalar.dma_start(
                                out=vtok[:, q * 4:(q + 1) * 4, kv * P:(kv + 1) * P],
                                in_=vrow, transpose=True)

                # ---------- Phase B/C: attention + o-proj ----------
                xmid = [midkeep.tile([P, E], f32, tag=f"xmid{i}", name=f"xmid{i}")
                        for i in range(4)]
                with tc.tile_pool(name="oT_keep", bufs=1) as oT_keep:
                    oT_all = oT_keep.tile([P, H, OWN], bf16, name="oT_all")
                    with (
                        tc.tile_pool(name="pB", bufs=1) as pB,
                        tc.tile_pool(name="psB", bufs=1, space="PSUM") as psB,
                    ):
                        masks = pB.tile([P, NT, OWN], bf16, name="masks")
                        nc.sync.dma_start(out=masks, in_=maskd)
                        # two heads sharing a kv head run interleaved so the
                        # PE->ACT->DVE->PE softmax round-trip of one hides the
                        # other; consecutive matmuls also share lhsT.
                        def emit_post(pend):
                            """Deferred bc/normalize of a finished head pair;
                            emitted a few iterations into the next pair so the
                            PE never waits on the reciprocal."""
                            hs_p, ps_o_p, rden_p = pend
                            for u in range(2):
                                ps_bc = psB.tile([P, OWN], f32, tag="ps_s",
                                                 bufs=4,
                                                 name=f"bc_ps_{hs_p[u]}")
                                nc.tensor.matmul(ps_bc, ones_row, rden_p[u],
                                                 start=True, stop=True)
                                bcs = pB.tile([P, OWN], f32, tag="bc", bufs=2,
                                              name=f"bc_{hs_p[u]}")
                                nc.vector.tensor_copy(bcs, ps_bc)
                                nc.vector.tensor_tensor(
                                    out=oT_all[:, hs_p[u], :], in0=ps_o_p[u],
                                    in1=bcs, op=mult)

                        pend = None
                        for g in range(H // 2):
                            hs = (2 * g, 2 * g + 1)
                            kv = hs[0] // G
                            ps_o = [psB.tile([P, OWN], f32, tag=f"ps_o{u}",
                                             bufs=2, name=f"ps_o_{g}_{u}")
                                    for u in range(2)]
                            ps_den = [psB.tile([1, OWN], f32, tag=f"ps_den{u}",
                                               bufs=1, name=f"ps_den_{g}_{u}")
                                      for u in range(2)]
                            for i in range(NT):
                                w = P * _alive(i)
                                if i == 4 and pend is not None:
                                    emit_post(pend)
                                    pend = None
                                ex = [None, None]
                                for u in range(2):
                                    ps_s = psB.tile([P, OWN], f32, tag="ps_s",
                                                    bufs=2,
                                                    name=f"ps_s_{g}_{i}_{u}")
                                    nc.tensor.matmul(ps_s[:, 0:w],
                                                     kT[kv][:, i * P:(i + 1) * P],
                                                     qT_all[:, hs[u], 0:w],
                                                     start=True, stop=True)
                                    ex[u] = pB.tile([P, OWN], bf16, tag="ex",
                                                    bufs=6,
                                                    name=f"ex_{g}_{i}_{u}")
                                    nc.scalar.activation(out=ex[u][:, 0:w],
                                                         in_=ps_s[:, 0:w],
                                                         func=Exp,
                                                         scale=EXP_SCALE)
                                    nc.vector.tensor_tensor(out=ex[u][:, 0:w],
                                                            in0=ex[u][:, 0:w],
                                                            in1=masks[:, i, 0:w],
                                                            op=mult)
                                for u in range(2):
                                    nc.tensor.matmul(ps_o[u][:, 0:w],
                                                     vtok[:, i, kv * P:(kv + 1) * P],
                                                     ex[u][:, 0:w],
                                                     start=(i == 0),
                                                     stop=(i == NT - 1))
                                for u in range(2):
                                    nc.tensor.matmul(ps_den[u][:, 0:w], ones_col,
                                                     ex[u][:, 0:w],
                                                     start=(i == 0),
                                                     stop=(i == NT - 1))
                            rden = [None, None]
                            for u in range(2):
                                rden[u] = pB.tile([1, OWN], bf16, tag="rden",
                                                  bufs=4, name=f"rden_{g}_{u}")
                                with nc.allow_low_precision(
                                        reason="softmax denominator"):
                                    nc.vector.reciprocal(out=rden[u],
                                                         in_=ps_den[u])
                            pend = (hs, ps_o, rden)
                        emit_post(pend)

                    with (
                        tc.tile_pool(name="pC", bufs=1) as pC,
                        tc.tile_pool(name="psC", bufs=1, space="PSUM") as psC,
                    ):
                        bo_b = pC.tile([P, E], f32, tag="bo_b")
                        nc.sync.dma_start(out=bo_b,
                                          in_=bo.unsqueeze(0).to_broadcast((P, E)))
                        for ms in range(4):
                            nc.scalar.dma_start(out=xmid[ms],
                                              in_=xow[ms * P:(ms + 1) * P, :])
                            nc.vector.tensor_tensor(out=xmid[ms], in0=xmid[ms],
                                                    in1=bo_b, op=add)
                        for ec in range(4):
                            pso = [psC.tile([P, 512], f32, tag=f"pso{i}", bufs=2,
                                            name=f"pso{i}") for i in range(4)]
                            for k in range(H):
                                wtile = pC.tile([P, 512], bf16, tag="wo", bufs=4)
                                nc.sync.dma_start(out=wtile, in_=wo_t[k, ec])
                                for ms in range(4):
                                    nc.tensor.matmul(
                                        pso[ms],
                                        oT_all[:, k, ms * P:(ms + 1) * P],
                                        wtile, start=(k == 0),
                                        stop=(k == H - 1))
                            for ms in range(4):
                                nc.vector.tensor_tensor(
                                    out=xmid[ms][:, ec * 512:(ec + 1) * 512],
                                    in0=pso[ms],
                                    in1=xmid[ms][:, ec * 512:(ec + 1) * 512], op=add)

            # ---------- Phase D/E: LN2, MLP ----------
            with (
                tc.tile_pool(name="pM", bufs=1) as pM,
                tc.tile_pool(name="psM", bufs=1, space="PSUM") as psM,
            ):
                bd_b = pM.tile([P, E], f32, tag="bd_b")
                nc.sync.dma_start(out=bd_b, in_=bd.unsqueeze(0).to_broadcast((P, E)))
                x2T = pM.tile([P, NE, OWN], bf16, name="x2T")
                for ms in range(4):
                    x2b = pM.tile([P, E], bf16, tag="x2b", bufs=2)
                    _ln_to_bf16(nc, pM, xmid[ms], eps_t, zero_t, x2b)
                    nc.scalar.dma_start(out=x2T[:, :, ms * P:(ms + 1) * P], in_=x2b,
                                      transpose=True)
                    # fold the final bias into the residual so the tail is
                    # a single add per chunk
                    nc.vector.tensor_tensor(out=xmid[ms], in0=xmid[ms],
                                            in1=bd_b, op=add)
                outp = [pM.tile([P, E], f32, tag=f"outp{i}", name=f"outp{i}")
                        for i in range(4)]
                hT = [pM.tile([P, OWN], bf16, tag=f"hT{i}", name=f"hT{i}")
                      for i in range(NF // 2)]
                for fh in range(2):
                    for fi in range(NF // 2):
                        f = fh * (NF // 2) + fi
                        wstrip = pM.tile([P, NE, P], bf16, tag="wu", bufs=3)
                        weng = nc.sync if fi % 2 == 0 else nc.scalar
                        weng.dma_start(out=wstrip, in_=wu_s[f])
                        psh = psM.tile([P, OWN], f32, tag="psh", bufs=2)
                        for e in range(NE):
                            nc.tensor.matmul(psh, wstrip[:, e, :], x2T[:, e, :],
                                             start=(e == 0), stop=(e == NE - 1))
                        nc.scalar.activation(out=hT[fi], in_=psh, func=Gelu,
                                             bias=bu_sb[:, f:f + 1])
                    for ec in range(4):
                        psd = [psM.tile([P, 512], f32, tag=f"psd{i}", bufs=1,
                                        name=f"psd{i}") for i in range(4)]
                        for fi in range(NF // 2):
                            f = fh * (NF // 2) + fi
                            wtile = pM.tile([P, 512], bf16, tag="wd", bufs=6)
                            weng = nc.sync if fi % 2 == 0 else nc.scalar
                            weng.dma_start(out=wtile, in_=wd_t[f, ec])
                            for ms in range(4):
                                nc.tensor.matmul(psd[ms],
                                                 hT[fi][:, ms * P:(ms + 1) * P],
                                                 wtile, start=(fi == 0),
                                                 stop=(fi == NF // 2 - 1))
                        cols = slice(ec * 512, (ec + 1) * 512)
                        for ms in range(4):
                            if fh == 0:
                                nc.vector.tensor_copy(outp[ms][:, cols], psd[ms])
                            else:
                                nc.vector.tensor_tensor(
                                    out=outp[ms][:, cols], in0=psd[ms],
                                    in1=outp[ms][:, cols], op=add)
                                nc.vector.tensor_tensor(
                                    out=outp[ms][:, cols], in0=outp[ms][:, cols],
                                    in1=xmid[ms][:, cols], op=add)
                                nc.sync.dma_start(
                                    out=out[ms * P:(ms + 1) * P, cols],
                                    in_=outp[ms][:, cols])


_NC_CACHE = None
LAST_RESULTS = None


def _get_nc():
    global _NC_CACHE
    if _NC_CACHE is None:
        nc = build()
        split_waits(nc)
        _NC_CACHE = nc
    return _NC_CACHE


def _slot_tiles(j):
    return [15 - j, 11 - j, j + 4, j]


def _prep_shared(wq, wk, wv, wo, wu, wd, g1, b1, g2, b2, bq, bk, bv, bu):
    """Pre-shard the weights; the LN affine transforms are folded in exactly:
    LN_g_b(x) @ W + b == LN_plain(x) @ (diag(g) W) + (b_ln @ W + b)."""
    def strips(w, n):  # [E, n*128] -> [n, 128(p), NE(t), 128(m)] contiguous
        a = np.asarray(w, np.float32).reshape(NE, P, n, P)
        return np.ascontiguousarray(a.transpose(2, 1, 0, 3)).astype(BF)

    def tiles(w, nr):  # [nr*128, E] -> [nr, 4, 128, 512]
        a = np.asarray(w, np.float32).reshape(nr, P, 4, 512)
        return np.ascontiguousarray(a.transpose(0, 2, 1, 3)).astype(BF)

    wq, wk, wv, wu = (np.asarray(w, np.float32) for w in (wq, wk, wv, wu))
    g1, b1, g2, b2 = (np.asarray(v, np.float32) for v in (g1, b1, g2, b2))
    wq1, wk1, wv1 = g1[:, None] * wq, g1[:, None] * wk, g1[:, None] * wv
    wu2 = g2[:, None] * wu
    return {
        "wq_s": strips(wq1, H),
        "wk_s": strips(wk1, KH),
        "wv_s": strips(wv1, KH),
        "wo_t": tiles(wo, H),
        "wu_s": strips(wu2, NF),
        "wd_t": tiles(wd, NF),
        "bq": np.asarray(bq, np.float32) + b1 @ wq,
        "bk": np.asarray(bk, np.float32) + b1 @ wk,
        "bv": np.asarray(bv, np.float32) + b1 @ wv,
        "bu": np.asarray(bu, np.float32) + b2 @ wu,
    }


def _make_masks(j):
    m = np.zeros((P, NT, OWN), np.float32)
    ki = np.arange(P)[:, None]
    qi = np.arange(P)[None, :]
    tri = (ki <= qi).astype(np.float32)
    for s, t in enumerate(_slot_tiles(j)):
        for i in range(CAPS[s]):
            if i < t:
                m[:, i, s * P:(s + 1) * P] = 1.0
            elif i == t:
                m[:, i, s * P:(s + 1) * P] = tri
    return m.astype(BF)


def kernel(x, ln1_g, ln1_b, wq, bq, wk, bk, wv, bv, wo, bo, ln2_g, ln2_b,
           wu, bu, wd, bd):
    x = np.asarray(x, np.float32)
    shared = _prep_shared(wq, wk, wv, wo, wu, wd, ln1_g, ln1_b, ln2_g, ln2_b,
                          bq, bk, bv, bu)
    shared.update({
        "bo": np.asarray(bo, np.float32), "bd": np.asarray(bd, np.float32),
    })
    in_maps = []
    for core in range(8):
        b, j = divmod(core, 4)
        m = dict(shared)
        m["xkv"] = np.ascontiguousarray(x[b])
        m["xow"] = np.ascontiguousarray(np.concatenate(
            [x[b, t * P:(t + 1) * P] for t in _slot_tiles(j)], axis=0))
        m["masks"] = _make_masks(j)
        in_maps.append(m)

    nc = _get_nc()
    trace = bool(os.environ.get("KERNEL_TRACE"))
    res = bass_utils.run_bass_kernel_spmd(
        nc, in_maps, core_ids=list(range(8)), trace=trace)
    global LAST_RESULTS
    LAST_RESULTS = res
    out = np.empty((B, S, E), np.float32)
    for core in range(8):
        b, j = divmod(core, 4)
        r = res.results[core]["out"]
        for s, t in enumerate(_slot_tiles(j)):
            out[b, t * P:(t + 1) * P] = r[s * P:(s + 1) * P]
    return out
